# revision 17
# baseline (speedup 1.0000x reference)
"""Causal self-attention (B=4, T=2048, D=1024, H=16) on 8 TRN2 NeuronCores.

Sharding: core c handles batch b = c//2 and head-group g = c%2 (8 heads each).
Each core computes, for its (b, g):
    qkv_loc = x[b] @ w_qkv[:, cols(g)]          (q|k|v local, 512 cols each)
    att     = causal_attention(q, k, v)          (8 heads, hd=64)
    y_part  = att @ w_out[rows(g), :]            ([2048, 1024] partial)
Host sums the two partial outputs per batch.

v2 structure (vs v1 phases A/B/C/D):
 - QK projection is interleaved per head-pair with that pair's attention so
   the TensorEngine stays dense (HAM stays warm) while ScalarE streams exps.
 - Scores emission is software-pipelined (scores chunk j+2 issued before the
   AV matmuls of chunk j) so the PE never queues behind an exp.
 - Softmax normalization: rowsums come free from a ones-column appended to V;
   1/rowsum via custom-DVE reciprocal_approx_fast (SBUF-only inputs!),
   partition-broadcast with a K=1 matmul, one DVE multiply. po ring has 3
   PSUM buffers so the norm chain never stalls the AV accumulation.
 - exp scores and V are bf16 (tolerance is 2e-2); Q/K/out-proj stay f32r.
 - Transpose evacuations batched 4-at-a-time; causal masks fused to 1024-wide.
"""

import numpy as np

import concourse.bass as bass
import concourse.mybir as mybir
from concourse import bacc, tile
from concourse import bass_utils
from concourse.masks import make_identity

B = 4
T = 2048
D = 1024
H = 16
HD = 64
H_LOC = 8
CLOC = H_LOC * HD       # 512
P = 128
N_CORES = 8

F32 = mybir.dt.float32
F32R = mybir.dt.float32r
BF16 = mybir.dt.bfloat16


def _build_kernel_body(nc, tc, x_ap, wqkv_ap, wout_ap, out_ap):
    from contextlib import ExitStack

    Exp = mybir.ActivationFunctionType.Exp
    Copy = mybir.ActivationFunctionType.Copy
    mult = mybir.AluOpType.mult

    def bitin(ap):
        return ap.bitcast(F32R)

    ctx = ExitStack()

    # ---------------- constants ----------------
    const = ctx.enter_context(tc.tile_pool(name="const", bufs=1))
    idscr = const.tile([P, P], F32, tag="idscr")
    make_identity(nc, idscr)
    ident = const.tile([P, P], F32R, tag="ident")
    nc.vector.tensor_copy(ident, idscr)
    ones_f = const.tile([P, P], F32, tag="ones")
    nc.gpsimd.memset(ones_f, 1.0)
    oc = const.tile([1, 64], F32R, tag="oc")
    nc.vector.tensor_copy(oc, ones_f[0:1, 0:64])
    zf = const.tile([64, 512], F32, tag="zf")
    nc.gpsimd.memset(zf, 0.0)

    # fused 1024-wide diagonal masks for the two chunks straddling the
    # diagonal: wmB[o][p, u*512 + c] = 1.0 iff p <= c - (o+u)*128
    wmB = {}
    for o in (0, 2):
        wt = const.tile([P, 1024], BF16, tag=f"wm{o}")
        nc.gpsimd.memset(wt, 1.0)
        for u in range(2):
            off = (o + u) * 128
            nc.gpsimd.affine_select(
                out=wt[:, u * 512:(u + 1) * 512],
                in_=wt[:, u * 512:(u + 1) * 512],
                compare_op=mybir.AluOpType.is_ge,  # keep where c - p - off >= 0
                fill=0.0,
                base=-off,
                channel_multiplier=-1,
                pattern=[[1, 512]],
            )
        wmB[o] = wt

    # ---------------- persistent tiles ----------------
    vat = ctx.enter_context(tc.tile_pool(name="vat", bufs=1))
    V_aug = vat.tile([P, 16, H_LOC, HD + 1], BF16, tag="vaug")
    nc.vector.tensor_copy(
        V_aug[:, :, :, HD], ones_f.rearrange("p (a b) -> p a b", a=16)
    )
    AT = vat.tile([P, 4, T], F32R, tag="AT")   # attention out, heads packed
    Qp0 = vat.tile([P, T], F32R, tag="qp0")    # padded Q, even head of pair
    Qp1 = vat.tile([P, T], F32R, tag="qp1")    # padded Q, odd head of pair

    xa = x_ap.rearrange("(tb p) d -> tb p d", p=P)          # [16, 128, 1024]
    wqk = wqkv_ap[:, 0:2 * CLOC].rearrange("(o p) c -> p o c", p=P)
    wv = wqkv_ap[:, 2 * CLOC:3 * CLOC].rearrange("(o p) c -> p o c", p=P)
    ch = CLOC // 2

    ldw = ctx.enter_context(tc.tile_pool(name="ldw", bufs=2))

    def load_w(s):
        wq = ldw.tile([P, 8, P], F32R, tag="wq")
        nc.sync.dma_start(wq, bitin(wqk[:, :, s * P:(s + 1) * P]))
        wk = ldw.tile([P, 8, P], F32R, tag="wk")
        nc.sync.dma_start(
            wk, bitin(wqk[:, :, CLOC + s * P:CLOC + (s + 1) * P])
        )
        return wq, wk

    w_pre = load_w(0)

    with tc.tile_pool(name="xt", bufs=1) as xt_pool:
        xT = xt_pool.tile([P, 8, T], F32R)   # [d%128, d//128, t]

        # ---- phase A: x -> xT (transpose), V projection (both halves) ----
        with tc.tile_pool(name="lda", bufs=2) as lda, \
             tc.tile_pool(name="ldv", bufs=1) as ldv, \
             tc.tile_pool(name="psA", bufs=2, space="PSUM") as psA:
            wv0 = ldv.tile([P, 8, ch], F32R, tag="wv0")
            nc.sync.dma_start(wv0, bitin(wv[:, :, 0:ch]))
            wv1 = ldv.tile([P, 8, ch], F32R, tag="wv1")
            nc.sync.dma_start(wv1, bitin(wv[:, :, ch:2 * ch]))
            for tb in range(T // P):
                xc = lda.tile([P, D], F32R, tag="xin")
                if tb < 2:  # gate of the first transposes: land in ~1/4 time
                    for q4 in range(4):
                        nc.sync.dma_start(
                            xc[:, q4 * 256:(q4 + 1) * 256],
                            bitin(xa[tb][:, q4 * 256:(q4 + 1) * 256]),
                        )
                else:
                    nc.sync.dma_start(xc, bitin(xa[tb]))
                for g in range(2):
                    pt4 = psA.tile([P, 512], F32R, tag="pt4")
                    for q in range(4):
                        db = g * 4 + q
                        nc.tensor.transpose(
                            pt4[:, q * P:(q + 1) * P],
                            xc[:, db * P:(db + 1) * P],
                            ident,
                        )
                    nc.vector.tensor_copy(
                        xT[:, g * 4:(g + 1) * 4, tb * P:(tb + 1) * P],
                        pt4.rearrange("p (q c) -> p q c", q=4),
                    )
                for half, wvt in ((0, wv0), (1, wv1)):
                    ps = psA.tile([P, ch], F32, tag="psv")
                    for k in range(8):
                        nc.tensor.matmul(
                            ps,
                            xT[:, k, tb * P:(tb + 1) * P],
                            wvt[:, k, :],
                            start=(k == 0),
                            stop=(k == 7),
                        )
                    nc.scalar.activation(
                        V_aug[:, tb, half * 4:(half + 1) * 4, 0:HD],
                        ps.rearrange("p (h d) -> p h d", h=4),
                        Copy,
                    )

        # zero the never-written Qp halves once (x*0 keeps f32r rounding legal)
        nc.vector.tensor_scalar_mul(Qp0[64:128, :], xT[64:128, 0, :], 0.0)
        nc.vector.tensor_scalar_mul(Qp1[0:64, :], xT[0:64, 0, :], 0.0)

        # ---- phases B+C interleaved: per head-pair QK proj + attention ----
        # PSUM budget: big 2x2 banks + po 3 + pb 1 = 8
        psM = ctx.enter_context(tc.tile_pool(name="psM", bufs=2, space="PSUM"))
        psPo = ctx.enter_context(tc.tile_pool(name="psPo", bufs=3, space="PSUM"))
        psPb = ctx.enter_context(tc.tile_pool(name="psPb", bufs=1, space="PSUM"))
        with tc.tile_pool(name="ktp", bufs=2) as ktp, \
             tc.tile_pool(name="esp", bufs=3) as esp, \
             tc.tile_pool(name="smp", bufs=3) as smp:

            pending = [None]

            def flush():
                if pending[0] is not None:
                    pending[0]()
                    pending[0] = None

            for s in range(4):
                # -- QK projection for heads (2s, 2s+1) --
                wq, wk = w_pre if s == 0 else load_w(s)
                KT = ktp.tile([P, T], F32R, tag="kt")
                for dst, wt in ((0, wq), (1, wk)):
                    for half in range(2):
                        t0 = half * 1024
                        ps = psM.tile([P, 1024], F32, tag="big")
                        for u in range(2):
                            for k in range(8):
                                nc.tensor.matmul(
                                    ps[:, u * 512:(u + 1) * 512],
                                    wt[:, k, :],
                                    xT[:, k, t0 + u * 512:t0 + (u + 1) * 512],
                                    start=(k == 0),
                                    stop=(k == 7),
                                )
                        if dst == 0:
                            nc.vector.tensor_copy(
                                Qp0[0:64, t0:t0 + 1024], ps[0:64, :]
                            )
                            nc.vector.tensor_copy(
                                Qp1[64:128, t0:t0 + 1024], ps[64:128, :]
                            )
                        else:
                            nc.vector.tensor_copy(KT[:, t0:t0 + 1024], ps)

                # -- attention for the pair --
                for hp in range(2):
                    h = 2 * s + hp
                    row0 = hp * 64
                    Qph = Qp0 if hp == 0 else Qp1
                    for it in range(4):
                        i0 = it * 512
                        njb = 4 * (it + 1)
                        nch = njb // 2
                        po = psPo.tile([P, 512], F32, tag="po")
                        sstiles = []

                        def emit_S(j, sstiles=sstiles, KT=KT, Qph=Qph, i0=i0):
                            ps = psM.tile([P, 1024], F32, tag="big")
                            for u in range(2):
                                jb = 2 * j + u
                                nc.tensor.matmul(
                                    ps[:, u * 512:(u + 1) * 512],
                                    KT[:, jb * P:(jb + 1) * P],
                                    Qph[:, i0:i0 + 512],
                                    start=True,
                                    stop=True,
                                )
                            sstiles.append(ps)

                        emit_S(0)
                        if nch > 1:
                            emit_S(1)
                        flush()
                        for j in range(nch):
                            ps = sstiles[j]
                            es = esp.tile([P, 1024], BF16, tag="es")
                            nc.scalar.activation(es, ps, Exp, scale=0.125)
                            off0 = 2 * j * P - i0
                            if off0 >= 0:  # chunk straddles the diagonal
                                nc.vector.tensor_tensor(
                                    es, es, wmB[off0 // P], mult
                                )
                            if j + 2 < nch:
                                emit_S(j + 2)
                            for u in range(2):
                                jb = 2 * j + u
                                nc.tensor.matmul(
                                    po[0:HD + 1, :],
                                    V_aug[:, jb, h, :],
                                    es[:, u * 512:(u + 1) * 512],
                                    start=(jb == 0),
                                    stop=(jb == njb - 1),
                                )

                        def mknorm(po=po, row0=row0, sub=s, i0=i0):
                            def norm():
                                rr = smp.tile([1, 512], F32, tag="rr")
                                nc.vector.tensor_copy(rr, po[HD:HD + 1, :])
                                ri = smp.tile([1, 512], F32, tag="ri")
                                nc.vector.reciprocal_approx_fast(out=ri, in_=rr)
                                rm = smp.tile([1, 512], F32R, tag="rm")
                                nc.vector.tensor_copy(rm, ri)
                                pb = psPb.tile([64, 512], F32, tag="pb")
                                nc.tensor.matmul(
                                    pb, oc, rm, start=True, stop=True,
                                )
                                rb = smp.tile([64, 512], F32, tag="rb")
                                nc.vector.tensor_copy(rb, pb)
                                nc.vector.tensor_tensor(
                                    AT[row0:row0 + 64, sub, i0:i0 + 512],
                                    po[0:HD, :],
                                    rb,
                                    mult,
                                )
                            return norm

                        pending[0] = mknorm()
            flush()

    # ---------------- phase D: output projection ----------------
    wo_v = wout_ap.rearrange("(o p) n -> p o n", p=P)  # [128, 4, 1024]
    oa = out_ap.rearrange("(tb p) d -> tb p d", p=P)
    with tc.tile_pool(name="ldo", bufs=1) as ldo, \
         tc.tile_pool(name="yp", bufs=3) as yp:
        wo = ldo.tile([P, 4, D], F32R, tag="wo")
        nc.sync.dma_start(wo, bitin(wo_v))
        for tb in range(T // P):
            py = psM.tile([P, 1024], F32, tag="big")
            for u in range(2):
                for k in range(4):
                    nc.tensor.matmul(
                        py[:, u * 512:(u + 1) * 512],
                        AT[:, k, tb * P:(tb + 1) * P],
                        wo[:, k, u * 512:(u + 1) * 512],
                        start=(k == 0),
                        stop=(k == 3),
                    )
            ysb = yp.tile([P, D], F32, tag="ysb")
            nc.scalar.activation(ysb, py, Copy)
            if tb == T // P - 1:  # tail: last store in ~1/4 time
                for q4 in range(4):
                    nc.sync.dma_start(
                        oa[tb][:, q4 * 256:(q4 + 1) * 256],
                        ysb[:, q4 * 256:(q4 + 1) * 256],
                    )
            else:
                nc.sync.dma_start(oa[tb], ysb)

    ctx.close()


_CACHE = {}

MM_MODE = "f32r"  # kept for test.py compat; v2 is f32r+bf16 mixed only


def _get_nc(mode=None):
    key = "v2"
    if key in _CACHE:
        return _CACHE[key]
    nc = bacc.Bacc(
        "TRN2",
        target_bir_lowering=False,
        debug=False,
        enable_asserts=False,
        num_devices=N_CORES,
    )
    x_d = nc.dram_tensor("x", [T, D], F32, kind="ExternalInput")
    wqkv_d = nc.dram_tensor("w_qkv", [D, 3 * CLOC], F32, kind="ExternalInput")
    wout_d = nc.dram_tensor("w_out", [CLOC, D], F32, kind="ExternalInput")
    out_d = nc.dram_tensor("out", [T, D], F32, kind="ExternalOutput")
    with tile.TileContext(nc) as tc:
        _build_kernel_body(
            nc, tc, x_d.ap(), wqkv_d.ap(), wout_d.ap(), out_d.ap()
        )
    nc.compile()
    _CACHE[key] = nc
    return nc


def _make_in_maps(x, w_qkv, w_out):
    x = np.ascontiguousarray(np.asarray(x, dtype=np.float32))
    w_qkv = np.ascontiguousarray(np.asarray(w_qkv, dtype=np.float32))
    w_out = np.ascontiguousarray(np.asarray(w_out, dtype=np.float32))
    in_maps = []
    for c in range(N_CORES):
        b, g = divmod(c, 2)
        c0 = g * CLOC
        wloc = np.concatenate(
            [
                w_qkv[:, c0:c0 + CLOC],
                w_qkv[:, D + c0:D + c0 + CLOC],
                w_qkv[:, 2 * D + c0:2 * D + c0 + CLOC],
            ],
            axis=1,
        )
        in_maps.append({
            "x": np.ascontiguousarray(x[b]),
            "w_qkv": np.ascontiguousarray(wloc),
            "w_out": np.ascontiguousarray(w_out[c0:c0 + CLOC]),
        })
    return in_maps


def run(x, w_qkv, w_out, trace=False, mode=None):
    nc = _get_nc(mode)
    in_maps = _make_in_maps(x, w_qkv, w_out)
    res = bass_utils.run_bass_kernel_spmd(
        nc, in_maps, core_ids=list(range(N_CORES)), trace=trace
    )
    y = np.empty((B, T, D), dtype=np.float32)
    for b in range(B):
        y[b] = res.results[2 * b]["out"] + res.results[2 * b + 1]["out"]
    return y, res


def kernel(x, w_qkv, w_out):
    y, _ = run(x, w_qkv, w_out, trace=False)
    return y


# revision 18
# speedup vs baseline: 1.0061x; 1.0061x over previous
"""Causal self-attention (B=4, T=2048, D=1024, H=16) on 8 TRN2 NeuronCores.

Sharding: core c handles batch b = c//2 and head-group g = c%2 (8 heads each).
Each core computes, for its (b, g):
    qkv_loc = x[b] @ w_qkv[:, cols(g)]          (q|k|v local, 512 cols each)
    att     = causal_attention(q, k, v)          (8 heads, hd=64)
    y_part  = att @ w_out[rows(g), :]            ([2048, 1024] partial)
Host sums the two partial outputs per batch.

v2 structure (vs v1 phases A/B/C/D):
 - QK projection is interleaved per head-pair with that pair's attention so
   the TensorEngine stays dense (HAM stays warm) while ScalarE streams exps.
 - Scores emission is software-pipelined (scores chunk j+2 issued before the
   AV matmuls of chunk j) so the PE never queues behind an exp.
 - Softmax normalization: rowsums come free from a ones-column appended to V;
   1/rowsum via custom-DVE reciprocal_approx_fast (SBUF-only inputs!),
   partition-broadcast with a K=1 matmul, one DVE multiply. po ring has 3
   PSUM buffers so the norm chain never stalls the AV accumulation.
 - exp scores and V are bf16 (tolerance is 2e-2); Q/K/out-proj stay f32r.
 - Transpose evacuations batched 4-at-a-time; causal masks fused to 1024-wide.
"""

import numpy as np

import concourse.bass as bass
import concourse.mybir as mybir
from concourse import bacc, tile
from concourse import bass_utils
from concourse.masks import make_identity

B = 4
T = 2048
D = 1024
H = 16
HD = 64
H_LOC = 8
CLOC = H_LOC * HD       # 512
P = 128
N_CORES = 8

F32 = mybir.dt.float32
F32R = mybir.dt.float32r
BF16 = mybir.dt.bfloat16


def _build_kernel_body(nc, tc, x_ap, wqkv_ap, wout_ap, out_ap):
    from contextlib import ExitStack

    Exp = mybir.ActivationFunctionType.Exp
    Copy = mybir.ActivationFunctionType.Copy
    mult = mybir.AluOpType.mult

    def bitin(ap):
        return ap.bitcast(F32R)

    ctx = ExitStack()

    # ---------------- constants ----------------
    const = ctx.enter_context(tc.tile_pool(name="const", bufs=1))
    idscr = const.tile([P, P], F32, tag="idscr")
    make_identity(nc, idscr)
    ident = const.tile([P, P], F32R, tag="ident")
    nc.vector.tensor_copy(ident, idscr)
    ones_f = const.tile([P, P], F32, tag="ones")
    nc.gpsimd.memset(ones_f, 1.0)
    oc = const.tile([1, 64], F32R, tag="oc")
    nc.vector.tensor_copy(oc, ones_f[0:1, 0:64])
    zf = const.tile([64, 512], F32, tag="zf")
    nc.gpsimd.memset(zf, 0.0)

    # fused 1024-wide diagonal masks for the two chunks straddling the
    # diagonal: wmB[o][p, u*512 + c] = 1.0 iff p <= c - (o+u)*128
    wmB = {}
    for o in (0, 2):
        wt = const.tile([P, 1024], BF16, tag=f"wm{o}")
        nc.gpsimd.memset(wt, 1.0)
        for u in range(2):
            off = (o + u) * 128
            nc.gpsimd.affine_select(
                out=wt[:, u * 512:(u + 1) * 512],
                in_=wt[:, u * 512:(u + 1) * 512],
                compare_op=mybir.AluOpType.is_ge,  # keep where c - p - off >= 0
                fill=0.0,
                base=-off,
                channel_multiplier=-1,
                pattern=[[1, 512]],
            )
        wmB[o] = wt

    # ---------------- persistent tiles ----------------
    vat = ctx.enter_context(tc.tile_pool(name="vat", bufs=1))
    V_aug = vat.tile([P, 16, H_LOC, HD + 1], BF16, tag="vaug")
    nc.vector.tensor_copy(
        V_aug[:, :, :, HD], ones_f.rearrange("p (a b) -> p a b", a=16)
    )
    AT = vat.tile([P, 4, T], F32R, tag="AT")   # attention out, heads packed
    Qp0 = vat.tile([P, T], F32R, tag="qp0")    # padded Q, even head of pair
    Qp1 = vat.tile([P, T], F32R, tag="qp1")    # padded Q, odd head of pair

    xa = x_ap.rearrange("(tb p) d -> tb p d", p=P)          # [16, 128, 1024]
    wqk = wqkv_ap[:, 0:2 * CLOC].rearrange("(o p) c -> p o c", p=P)
    wv = wqkv_ap[:, 2 * CLOC:3 * CLOC].rearrange("(o p) c -> p o c", p=P)
    ch = CLOC // 2

    ldw = ctx.enter_context(tc.tile_pool(name="ldw", bufs=2))

    def load_w(s):
        wq = ldw.tile([P, 8, P], F32R, tag="wq")
        nc.sync.dma_start(wq, bitin(wqk[:, :, s * P:(s + 1) * P]))
        wk = ldw.tile([P, 8, P], F32R, tag="wk")
        nc.sync.dma_start(
            wk, bitin(wqk[:, :, CLOC + s * P:CLOC + (s + 1) * P])
        )
        return wq, wk

    w_pre = load_w(0)

    with tc.tile_pool(name="xt", bufs=1) as xt_pool:
        xT = xt_pool.tile([P, 8, T], F32R)   # [d%128, d//128, t]

        # ---- phase A: x -> xT (transpose), V projection (both halves) ----
        with tc.tile_pool(name="lda", bufs=2) as lda, \
             tc.tile_pool(name="ldv", bufs=1) as ldv, \
             tc.tile_pool(name="psA", bufs=2, space="PSUM") as psA:
            wv0 = ldv.tile([P, 8, ch], F32R, tag="wv0")
            nc.sync.dma_start(wv0, bitin(wv[:, :, 0:ch]))
            wv1 = ldv.tile([P, 8, ch], F32R, tag="wv1")
            nc.sync.dma_start(wv1, bitin(wv[:, :, ch:2 * ch]))
            for tb in range(T // P):
                xc = lda.tile([P, D], F32R, tag="xin")
                if tb < 4:  # gate of the first transposes: land in ~1/4 time
                    for q4 in range(4):
                        nc.sync.dma_start(
                            xc[:, q4 * 256:(q4 + 1) * 256],
                            bitin(xa[tb][:, q4 * 256:(q4 + 1) * 256]),
                        )
                else:
                    nc.sync.dma_start(xc, bitin(xa[tb]))
                for g in range(2):
                    pt4 = psA.tile([P, 512], F32R, tag="pt4")
                    for q in range(4):
                        db = g * 4 + q
                        nc.tensor.transpose(
                            pt4[:, q * P:(q + 1) * P],
                            xc[:, db * P:(db + 1) * P],
                            ident,
                        )
                    nc.vector.tensor_copy(
                        xT[:, g * 4:(g + 1) * 4, tb * P:(tb + 1) * P],
                        pt4.rearrange("p (q c) -> p q c", q=4),
                    )
                for half, wvt in ((0, wv0), (1, wv1)):
                    ps = psA.tile([P, ch], F32, tag="psv")
                    for k in range(8):
                        nc.tensor.matmul(
                            ps,
                            xT[:, k, tb * P:(tb + 1) * P],
                            wvt[:, k, :],
                            start=(k == 0),
                            stop=(k == 7),
                        )
                    nc.scalar.activation(
                        V_aug[:, tb, half * 4:(half + 1) * 4, 0:HD],
                        ps.rearrange("p (h d) -> p h d", h=4),
                        Copy,
                    )

        # zero the never-written Qp halves once (x*0 keeps f32r rounding legal)
        nc.vector.tensor_scalar_mul(Qp0[64:128, :], xT[64:128, 0, :], 0.0)
        nc.vector.tensor_scalar_mul(Qp1[0:64, :], xT[0:64, 0, :], 0.0)

        # ---- phases B+C interleaved: per head-pair QK proj + attention ----
        # PSUM budget: big 2x2 banks + po 3 + pb 1 = 8
        psM = ctx.enter_context(tc.tile_pool(name="psM", bufs=2, space="PSUM"))
        psPo = ctx.enter_context(tc.tile_pool(name="psPo", bufs=3, space="PSUM"))
        psPb = ctx.enter_context(tc.tile_pool(name="psPb", bufs=1, space="PSUM"))
        with tc.tile_pool(name="ktp", bufs=2) as ktp, \
             tc.tile_pool(name="esp", bufs=3) as esp, \
             tc.tile_pool(name="smp", bufs=3) as smp:

            pending = [None]

            def flush():
                if pending[0] is not None:
                    pending[0]()
                    pending[0] = None

            for s in range(4):
                # -- QK projection for heads (2s, 2s+1) --
                wq, wk = w_pre if s == 0 else load_w(s)
                KT = ktp.tile([P, T], F32R, tag="kt")
                for dst, wt in ((0, wq), (1, wk)):
                    for half in range(2):
                        t0 = half * 1024
                        ps = psM.tile([P, 1024], F32, tag="big")
                        for u in range(2):
                            for k in range(8):
                                nc.tensor.matmul(
                                    ps[:, u * 512:(u + 1) * 512],
                                    wt[:, k, :],
                                    xT[:, k, t0 + u * 512:t0 + (u + 1) * 512],
                                    start=(k == 0),
                                    stop=(k == 7),
                                )
                        if dst == 0:
                            nc.vector.tensor_copy(
                                Qp0[0:64, t0:t0 + 1024], ps[0:64, :]
                            )
                            nc.vector.tensor_copy(
                                Qp1[64:128, t0:t0 + 1024], ps[64:128, :]
                            )
                        else:
                            nc.vector.tensor_copy(KT[:, t0:t0 + 1024], ps)

                # -- attention for the pair --
                for hp in range(2):
                    h = 2 * s + hp
                    row0 = hp * 64
                    Qph = Qp0 if hp == 0 else Qp1
                    for it in range(4):
                        i0 = it * 512
                        njb = 4 * (it + 1)
                        nch = njb // 2
                        po = psPo.tile([P, 512], F32, tag="po")
                        sstiles = []

                        def emit_S(j, sstiles=sstiles, KT=KT, Qph=Qph, i0=i0):
                            ps = psM.tile([P, 1024], F32, tag="big")
                            for u in range(2):
                                jb = 2 * j + u
                                nc.tensor.matmul(
                                    ps[:, u * 512:(u + 1) * 512],
                                    KT[:, jb * P:(jb + 1) * P],
                                    Qph[:, i0:i0 + 512],
                                    start=True,
                                    stop=True,
                                )
                            sstiles.append(ps)

                        emit_S(0)
                        if nch > 1:
                            emit_S(1)
                        flush()
                        for j in range(nch):
                            ps = sstiles[j]
                            es = esp.tile([P, 1024], BF16, tag="es")
                            nc.scalar.activation(es, ps, Exp, scale=0.125)
                            off0 = 2 * j * P - i0
                            if off0 >= 0:  # chunk straddles the diagonal
                                nc.vector.tensor_tensor(
                                    es, es, wmB[off0 // P], mult
                                )
                            if j + 2 < nch:
                                emit_S(j + 2)
                            for u in range(2):
                                jb = 2 * j + u
                                nc.tensor.matmul(
                                    po[0:HD + 1, :],
                                    V_aug[:, jb, h, :],
                                    es[:, u * 512:(u + 1) * 512],
                                    start=(jb == 0),
                                    stop=(jb == njb - 1),
                                )

                        def mknorm(po=po, row0=row0, sub=s, i0=i0):
                            def norm():
                                rr = smp.tile([1, 512], F32, tag="rr")
                                nc.vector.tensor_copy(rr, po[HD:HD + 1, :])
                                ri = smp.tile([1, 512], F32, tag="ri")
                                nc.vector.reciprocal_approx_fast(out=ri, in_=rr)
                                rm = smp.tile([1, 512], F32R, tag="rm")
                                nc.vector.tensor_copy(rm, ri)
                                pb = psPb.tile([64, 512], F32, tag="pb")
                                nc.tensor.matmul(
                                    pb, oc, rm, start=True, stop=True,
                                )
                                rb = smp.tile([64, 512], F32, tag="rb")
                                nc.vector.tensor_copy(rb, pb)
                                nc.vector.tensor_tensor(
                                    AT[row0:row0 + 64, sub, i0:i0 + 512],
                                    po[0:HD, :],
                                    rb,
                                    mult,
                                )
                            return norm

                        pending[0] = mknorm()
            flush()

    # ---------------- phase D: output projection ----------------
    wo_v = wout_ap.rearrange("(o p) n -> p o n", p=P)  # [128, 4, 1024]
    oa = out_ap.rearrange("(tb p) d -> tb p d", p=P)
    with tc.tile_pool(name="ldo", bufs=1) as ldo, \
         tc.tile_pool(name="yp", bufs=3) as yp:
        wo = ldo.tile([P, 4, D], F32R, tag="wo")
        nc.sync.dma_start(wo, bitin(wo_v))
        for tb in range(T // P):
            py = psM.tile([P, 1024], F32, tag="big")
            for u in range(2):
                for k in range(4):
                    nc.tensor.matmul(
                        py[:, u * 512:(u + 1) * 512],
                        AT[:, k, tb * P:(tb + 1) * P],
                        wo[:, k, u * 512:(u + 1) * 512],
                        start=(k == 0),
                        stop=(k == 3),
                    )
            ysb = yp.tile([P, D], F32, tag="ysb")
            nc.scalar.activation(ysb, py, Copy)
            if tb >= T // P - 4:  # tail: last stores in ~1/4 time
                for q4 in range(4):
                    nc.sync.dma_start(
                        oa[tb][:, q4 * 256:(q4 + 1) * 256],
                        ysb[:, q4 * 256:(q4 + 1) * 256],
                    )
            else:
                nc.sync.dma_start(oa[tb], ysb)

    ctx.close()


_CACHE = {}

MM_MODE = "f32r"  # kept for test.py compat; v2 is f32r+bf16 mixed only


def _get_nc(mode=None):
    key = "v2"
    if key in _CACHE:
        return _CACHE[key]
    nc = bacc.Bacc(
        "TRN2",
        target_bir_lowering=False,
        debug=False,
        enable_asserts=False,
        num_devices=N_CORES,
    )
    x_d = nc.dram_tensor("x", [T, D], F32, kind="ExternalInput")
    wqkv_d = nc.dram_tensor("w_qkv", [D, 3 * CLOC], F32, kind="ExternalInput")
    wout_d = nc.dram_tensor("w_out", [CLOC, D], F32, kind="ExternalInput")
    out_d = nc.dram_tensor("out", [T, D], F32, kind="ExternalOutput")
    with tile.TileContext(nc) as tc:
        _build_kernel_body(
            nc, tc, x_d.ap(), wqkv_d.ap(), wout_d.ap(), out_d.ap()
        )
    nc.compile()
    _CACHE[key] = nc
    return nc


def _make_in_maps(x, w_qkv, w_out):
    x = np.ascontiguousarray(np.asarray(x, dtype=np.float32))
    w_qkv = np.ascontiguousarray(np.asarray(w_qkv, dtype=np.float32))
    w_out = np.ascontiguousarray(np.asarray(w_out, dtype=np.float32))
    in_maps = []
    for c in range(N_CORES):
        b, g = divmod(c, 2)
        c0 = g * CLOC
        wloc = np.concatenate(
            [
                w_qkv[:, c0:c0 + CLOC],
                w_qkv[:, D + c0:D + c0 + CLOC],
                w_qkv[:, 2 * D + c0:2 * D + c0 + CLOC],
            ],
            axis=1,
        )
        in_maps.append({
            "x": np.ascontiguousarray(x[b]),
            "w_qkv": np.ascontiguousarray(wloc),
            "w_out": np.ascontiguousarray(w_out[c0:c0 + CLOC]),
        })
    return in_maps


def run(x, w_qkv, w_out, trace=False, mode=None):
    nc = _get_nc(mode)
    in_maps = _make_in_maps(x, w_qkv, w_out)
    res = bass_utils.run_bass_kernel_spmd(
        nc, in_maps, core_ids=list(range(N_CORES)), trace=trace
    )
    y = np.empty((B, T, D), dtype=np.float32)
    for b in range(B):
        y[b] = res.results[2 * b]["out"] + res.results[2 * b + 1]["out"]
    return y, res


def kernel(x, w_qkv, w_out):
    y, _ = run(x, w_qkv, w_out, trace=False)
    return y


# revision 19
# speedup vs baseline: 1.0181x; 1.0118x over previous
"""Causal self-attention (B=4, T=2048, D=1024, H=16) on 8 TRN2 NeuronCores.

Sharding: core c handles batch b = c//2 and head-group g = c%2 (8 heads each).
Each core computes, for its (b, g):
    qkv_loc = x[b] @ w_qkv[:, cols(g)]          (q|k|v local, 512 cols each)
    att     = causal_attention(q, k, v)          (8 heads, hd=64)
    y_part  = att @ w_out[rows(g), :]            ([2048, 1024] partial)
Host sums the two partial outputs per batch.

v2 structure (vs v1 phases A/B/C/D):
 - QK projection is interleaved per head-pair with that pair's attention so
   the TensorEngine stays dense (HAM stays warm) while ScalarE streams exps.
 - Scores emission is software-pipelined (scores chunk j+2 issued before the
   AV matmuls of chunk j) so the PE never queues behind an exp.
 - Softmax normalization: rowsums come free from a ones-column appended to V;
   1/rowsum via custom-DVE reciprocal_approx_fast (SBUF-only inputs!),
   partition-broadcast with a K=1 matmul, one DVE multiply. po ring has 3
   PSUM buffers so the norm chain never stalls the AV accumulation.
 - exp scores and V are bf16 (tolerance is 2e-2); Q/K/out-proj stay f32r.
 - Transpose evacuations batched 4-at-a-time; causal masks fused to 1024-wide.
"""

import numpy as np

import concourse.bass as bass
import concourse.mybir as mybir
from concourse import bacc, tile
from concourse import bass_utils
from concourse.masks import make_identity

B = 4
T = 2048
D = 1024
H = 16
HD = 64
H_LOC = 8
CLOC = H_LOC * HD       # 512
P = 128
N_CORES = 8

F32 = mybir.dt.float32
F32R = mybir.dt.float32r
BF16 = mybir.dt.bfloat16


def _build_kernel_body(nc, tc, x_ap, wqkv_ap, wout_ap, out_ap):
    from contextlib import ExitStack

    Exp = mybir.ActivationFunctionType.Exp
    Copy = mybir.ActivationFunctionType.Copy
    mult = mybir.AluOpType.mult

    def bitin(ap):
        return ap.bitcast(F32R)

    ctx = ExitStack()

    # ---------------- constants ----------------
    const = ctx.enter_context(tc.tile_pool(name="const", bufs=1))
    idscr = const.tile([P, P], F32, tag="idscr")
    make_identity(nc, idscr)
    ident = const.tile([P, P], F32R, tag="ident")
    nc.vector.tensor_copy(ident, idscr)
    ones_f = const.tile([P, P], F32, tag="ones")
    nc.gpsimd.memset(ones_f, 1.0)
    oc = const.tile([1, 64], F32R, tag="oc")
    nc.vector.tensor_copy(oc, ones_f[0:1, 0:64])
    zf = const.tile([64, 512], F32, tag="zf")
    nc.gpsimd.memset(zf, 0.0)

    # fused 1024-wide diagonal masks for the two chunks straddling the
    # diagonal: wmB[o][p, u*512 + c] = 1.0 iff p <= c - (o+u)*128
    wmB = {}
    for o in (0, 2):
        wt = const.tile([P, 1024], BF16, tag=f"wm{o}")
        nc.gpsimd.memset(wt, 1.0)
        for u in range(2):
            off = (o + u) * 128
            nc.gpsimd.affine_select(
                out=wt[:, u * 512:(u + 1) * 512],
                in_=wt[:, u * 512:(u + 1) * 512],
                compare_op=mybir.AluOpType.is_ge,  # keep where c - p - off >= 0
                fill=0.0,
                base=-off,
                channel_multiplier=-1,
                pattern=[[1, 512]],
            )
        wmB[o] = wt

    # ---------------- persistent tiles ----------------
    vat = ctx.enter_context(tc.tile_pool(name="vat", bufs=1))
    V_aug = vat.tile([P, 16, H_LOC, HD + 1], BF16, tag="vaug")
    nc.vector.tensor_copy(
        V_aug[:, :, :, HD], ones_f.rearrange("p (a b) -> p a b", a=16)
    )
    AT = vat.tile([P, 4, T], F32R, tag="AT")   # attention out, heads packed
    Qp0 = vat.tile([P, T], F32R, tag="qp0")    # padded Q, even head of pair
    Qp1 = vat.tile([P, T], F32R, tag="qp1")    # padded Q, odd head of pair

    xa = x_ap.rearrange("(tb p) d -> tb p d", p=P)          # [16, 128, 1024]
    wqk = wqkv_ap[:, 0:2 * CLOC].rearrange("(o p) c -> p o c", p=P)
    wv = wqkv_ap[:, 2 * CLOC:3 * CLOC].rearrange("(o p) c -> p o c", p=P)
    ch = CLOC // 2

    ldw = ctx.enter_context(tc.tile_pool(name="ldw", bufs=2))

    def load_w(s):
        wq = ldw.tile([P, 8, P], F32R, tag="wq")
        nc.sync.dma_start(wq, bitin(wqk[:, :, s * P:(s + 1) * P]))
        wk = ldw.tile([P, 8, P], F32R, tag="wk")
        nc.sync.dma_start(
            wk, bitin(wqk[:, :, CLOC + s * P:CLOC + (s + 1) * P])
        )
        return wq, wk

    w_pre = load_w(0)

    with tc.tile_pool(name="xt", bufs=1) as xt_pool:
        xT = xt_pool.tile([P, 8, T], F32R)   # [d%128, d//128, t]

        # ---- phase A: x -> xT (transpose), V projection (both halves) ----
        with tc.tile_pool(name="lda", bufs=4) as lda, \
             tc.tile_pool(name="ldv", bufs=1) as ldv, \
             tc.tile_pool(name="psA", bufs=2, space="PSUM") as psA:
            wv0 = ldv.tile([P, 8, ch], F32R, tag="wv0")
            nc.sync.dma_start(wv0, bitin(wv[:, :, 0:ch]))
            wv1 = ldv.tile([P, 8, ch], F32R, tag="wv1")
            nc.sync.dma_start(wv1, bitin(wv[:, :, ch:2 * ch]))
            for tb in range(T // P):
                xc = lda.tile([P, D], F32R, tag="xin")
                if tb < 4:  # gate of the first transposes: land in ~1/4 time
                    for q4 in range(4):
                        nc.sync.dma_start(
                            xc[:, q4 * 256:(q4 + 1) * 256],
                            bitin(xa[tb][:, q4 * 256:(q4 + 1) * 256]),
                        )
                else:
                    nc.sync.dma_start(xc, bitin(xa[tb]))
                for g in range(2):
                    pt4 = psA.tile([P, 512], F32R, tag="pt4")
                    for q in range(4):
                        db = g * 4 + q
                        nc.tensor.transpose(
                            pt4[:, q * P:(q + 1) * P],
                            xc[:, db * P:(db + 1) * P],
                            ident,
                        )
                    nc.vector.tensor_copy(
                        xT[:, g * 4:(g + 1) * 4, tb * P:(tb + 1) * P],
                        pt4.rearrange("p (q c) -> p q c", q=4),
                    )
                for half, wvt in ((0, wv0), (1, wv1)):
                    ps = psA.tile([P, ch], F32, tag="psv")
                    for k in range(8):
                        nc.tensor.matmul(
                            ps,
                            xT[:, k, tb * P:(tb + 1) * P],
                            wvt[:, k, :],
                            start=(k == 0),
                            stop=(k == 7),
                        )
                    nc.scalar.activation(
                        V_aug[:, tb, half * 4:(half + 1) * 4, 0:HD],
                        ps.rearrange("p (h d) -> p h d", h=4),
                        Copy,
                    )

        # zero the never-written Qp halves once (x*0 keeps f32r rounding legal)
        nc.vector.tensor_scalar_mul(Qp0[64:128, :], xT[64:128, 0, :], 0.0)
        nc.vector.tensor_scalar_mul(Qp1[0:64, :], xT[0:64, 0, :], 0.0)

        # ---- phases B+C interleaved: per head-pair QK proj + attention ----
        # PSUM budget: big 2x2 banks + po 3 + pb 1 = 8
        psM = ctx.enter_context(tc.tile_pool(name="psM", bufs=2, space="PSUM"))
        psPo = ctx.enter_context(tc.tile_pool(name="psPo", bufs=3, space="PSUM"))
        psPb = ctx.enter_context(tc.tile_pool(name="psPb", bufs=1, space="PSUM"))
        with tc.tile_pool(name="ktp", bufs=2) as ktp, \
             tc.tile_pool(name="esp", bufs=4) as esp, \
             tc.tile_pool(name="smp", bufs=3) as smp:

            pending = [None]

            def flush():
                if pending[0] is not None:
                    pending[0]()
                    pending[0] = None

            for s in range(4):
                # -- QK projection for heads (2s, 2s+1) --
                wq, wk = w_pre if s == 0 else load_w(s)
                KT = ktp.tile([P, T], F32R, tag="kt")
                for dst, wt in ((0, wq), (1, wk)):
                    for half in range(2):
                        t0 = half * 1024
                        ps = psM.tile([P, 1024], F32, tag="big")
                        for u in range(2):
                            for k in range(8):
                                nc.tensor.matmul(
                                    ps[:, u * 512:(u + 1) * 512],
                                    wt[:, k, :],
                                    xT[:, k, t0 + u * 512:t0 + (u + 1) * 512],
                                    start=(k == 0),
                                    stop=(k == 7),
                                )
                        if dst == 0:
                            nc.vector.tensor_copy(
                                Qp0[0:64, t0:t0 + 1024], ps[0:64, :]
                            )
                            nc.vector.tensor_copy(
                                Qp1[64:128, t0:t0 + 1024], ps[64:128, :]
                            )
                        else:
                            nc.vector.tensor_copy(KT[:, t0:t0 + 1024], ps)

                # -- attention for the pair --
                for hp in range(2):
                    h = 2 * s + hp
                    row0 = hp * 64
                    Qph = Qp0 if hp == 0 else Qp1
                    for it in range(4):
                        i0 = it * 512
                        njb = 4 * (it + 1)
                        nch = njb // 2
                        po = psPo.tile([P, 512], F32, tag="po")
                        sstiles = []

                        def emit_S(j, sstiles=sstiles, KT=KT, Qph=Qph, i0=i0):
                            ps = psM.tile([P, 1024], F32, tag="big")
                            for u in range(2):
                                jb = 2 * j + u
                                nc.tensor.matmul(
                                    ps[:, u * 512:(u + 1) * 512],
                                    KT[:, jb * P:(jb + 1) * P],
                                    Qph[:, i0:i0 + 512],
                                    start=True,
                                    stop=True,
                                )
                            sstiles.append(ps)

                        emit_S(0)
                        if nch > 1:
                            emit_S(1)
                        flush()
                        for j in range(nch):
                            ps = sstiles[j]
                            es = esp.tile([P, 1024], BF16, tag="es")
                            nc.scalar.activation(es, ps, Exp, scale=0.125)
                            off0 = 2 * j * P - i0
                            if off0 >= 0:  # chunk straddles the diagonal
                                nc.vector.tensor_tensor(
                                    es, es, wmB[off0 // P], mult
                                )
                            if j + 2 < nch:
                                emit_S(j + 2)
                            for u in range(2):
                                jb = 2 * j + u
                                nc.tensor.matmul(
                                    po[0:HD + 1, :],
                                    V_aug[:, jb, h, :],
                                    es[:, u * 512:(u + 1) * 512],
                                    start=(jb == 0),
                                    stop=(jb == njb - 1),
                                )

                        def mknorm(po=po, row0=row0, sub=s, i0=i0):
                            def norm():
                                rr = smp.tile([1, 512], F32, tag="rr")
                                nc.vector.tensor_copy(rr, po[HD:HD + 1, :])
                                ri = smp.tile([1, 512], F32, tag="ri")
                                nc.vector.reciprocal_approx_fast(out=ri, in_=rr)
                                rm = smp.tile([1, 512], F32R, tag="rm")
                                nc.vector.tensor_copy(rm, ri)
                                pb = psPb.tile([64, 512], F32, tag="pb")
                                nc.tensor.matmul(
                                    pb, oc, rm, start=True, stop=True,
                                )
                                rb = smp.tile([64, 512], F32, tag="rb")
                                nc.vector.tensor_copy(rb, pb)
                                nc.vector.tensor_tensor(
                                    AT[row0:row0 + 64, sub, i0:i0 + 512],
                                    po[0:HD, :],
                                    rb,
                                    mult,
                                )
                            return norm

                        pending[0] = mknorm()
            flush()

    # ---------------- phase D: output projection ----------------
    wo_v = wout_ap.rearrange("(o p) n -> p o n", p=P)  # [128, 4, 1024]
    oa = out_ap.rearrange("(tb p) d -> tb p d", p=P)
    with tc.tile_pool(name="ldo", bufs=1) as ldo, \
         tc.tile_pool(name="yp", bufs=3) as yp:
        wo = ldo.tile([P, 4, D], F32R, tag="wo")
        nc.sync.dma_start(wo, bitin(wo_v))
        for tb in range(T // P):
            py = psM.tile([P, 1024], F32, tag="big")
            for u in range(2):
                for k in range(4):
                    nc.tensor.matmul(
                        py[:, u * 512:(u + 1) * 512],
                        AT[:, k, tb * P:(tb + 1) * P],
                        wo[:, k, u * 512:(u + 1) * 512],
                        start=(k == 0),
                        stop=(k == 3),
                    )
            ysb = yp.tile([P, D], F32, tag="ysb")
            nc.scalar.activation(ysb, py, Copy)
            if tb >= T // P - 4:  # tail: last stores in ~1/4 time
                for q4 in range(4):
                    nc.sync.dma_start(
                        oa[tb][:, q4 * 256:(q4 + 1) * 256],
                        ysb[:, q4 * 256:(q4 + 1) * 256],
                    )
            else:
                nc.sync.dma_start(oa[tb], ysb)

    ctx.close()


_CACHE = {}

MM_MODE = "f32r"  # kept for test.py compat; v2 is f32r+bf16 mixed only


def _get_nc(mode=None):
    key = "v2"
    if key in _CACHE:
        return _CACHE[key]
    nc = bacc.Bacc(
        "TRN2",
        target_bir_lowering=False,
        debug=False,
        enable_asserts=False,
        num_devices=N_CORES,
    )
    x_d = nc.dram_tensor("x", [T, D], F32, kind="ExternalInput")
    wqkv_d = nc.dram_tensor("w_qkv", [D, 3 * CLOC], F32, kind="ExternalInput")
    wout_d = nc.dram_tensor("w_out", [CLOC, D], F32, kind="ExternalInput")
    out_d = nc.dram_tensor("out", [T, D], F32, kind="ExternalOutput")
    with tile.TileContext(nc) as tc:
        _build_kernel_body(
            nc, tc, x_d.ap(), wqkv_d.ap(), wout_d.ap(), out_d.ap()
        )
    nc.compile()
    _CACHE[key] = nc
    return nc


def _make_in_maps(x, w_qkv, w_out):
    x = np.ascontiguousarray(np.asarray(x, dtype=np.float32))
    w_qkv = np.ascontiguousarray(np.asarray(w_qkv, dtype=np.float32))
    w_out = np.ascontiguousarray(np.asarray(w_out, dtype=np.float32))
    in_maps = []
    for c in range(N_CORES):
        b, g = divmod(c, 2)
        c0 = g * CLOC
        wloc = np.concatenate(
            [
                w_qkv[:, c0:c0 + CLOC],
                w_qkv[:, D + c0:D + c0 + CLOC],
                w_qkv[:, 2 * D + c0:2 * D + c0 + CLOC],
            ],
            axis=1,
        )
        in_maps.append({
            "x": np.ascontiguousarray(x[b]),
            "w_qkv": np.ascontiguousarray(wloc),
            "w_out": np.ascontiguousarray(w_out[c0:c0 + CLOC]),
        })
    return in_maps


def run(x, w_qkv, w_out, trace=False, mode=None):
    nc = _get_nc(mode)
    in_maps = _make_in_maps(x, w_qkv, w_out)
    res = bass_utils.run_bass_kernel_spmd(
        nc, in_maps, core_ids=list(range(N_CORES)), trace=trace
    )
    y = np.empty((B, T, D), dtype=np.float32)
    for b in range(B):
        y[b] = res.results[2 * b]["out"] + res.results[2 * b + 1]["out"]
    return y, res


def kernel(x, w_qkv, w_out):
    y, _ = run(x, w_qkv, w_out, trace=False)
    return y
